# revision 72
# baseline (speedup 1.0000x reference)
"""Swin-style transformer block on 8 Trainium2 NeuronCores.

Sharding: data-parallel over batch - each of the 8 cores processes one image
([4096, 768] tokens). All weights replicated per core. No collectives.

v3 design (vs v2):
  - Attention GEMMs in fp8e4m3 with DoubleRow (qkv, proj, AV, den), scores in
    fp8 at 1 cycle/row with window-local moving dims (no cross-window waste).
  - Scores computed TRANSPOSED (S^T[k,q] per head/window): exp writes U^T fp8
    directly; rel-pos bias is added into the scores PSUM by a PE
    identity-matmul (128*B, exp scale=1/128). Edge windows (wi==4 / wj==4)
    stream only their valid query columns (packed in PSUM, exp scatters back
    to window coords); garbage in pad-q lanes only ever reaches unscattered
    pad-token rows.
  - AV contracts both kc chunks of a head-pair in K=256 DoubleRow matmuls
    against a checkerboard vT layout (even heads slab 0 / odd heads slab 1,
    complementary halves zero), 128-row outputs at partition 0 (ISA rule).
  - Softmax denominator: per-head K=128/M=128 DoubleRow selector-matmuls
    accumulate all heads into one PSUM tile per window; one DVE reciprocal;
    per head-pair a selector-matmul replicates 1/den, applied on the OT
    evacuation in one DVE multiply.
  - MLP entirely fp8 DoubleRow (fc1 x4/x4 scaling, gelu(ps/16+b) on evac;
    fc2 x16 with 1/16 folded into the residual DVE op), 512-token
    super-tiles, full 3072 hidden resident in SBUF.
  - MLP super-tiles are emitted as fine-grained generator units PUMPED into
    the attention pair loop (in-order engine queues then fill attention's
    exp/dependency bubbles with MLP work). Units carrying gelus are only
    pumped at points away from exp bursts (exp and gelu live in different
    ACT tables; 1283ns load per switch). LN1 prep for the next pair is
    likewise a generator pumped from inside stage 1 (DVE stats overlap ACT
    exp bursts). x2/out DMAs ride queues whose waits can't block compute
    (a DMA's dependency waits hold the issuing engine's queue).
  - LN rstd via batched DVE quake+Newton rsqrt (no Ln/Exp table funcs).
    x is fed from host as bf16 (halves gather bytes; residual quantization
    is equivalent to the existing bf16 x2 roundtrip).
"""

import numpy as np
from contextlib import ExitStack

import ml_dtypes

import concourse.bass as bass
import concourse.mybir as mybir
import concourse.tile as tile
from concourse import bacc
from concourse.bass_utils import run_bass_kernel_spmd
from concourse.masks import make_identity

F32 = mybir.dt.float32
F32R = mybir.dt.float32r
BF16 = mybir.dt.bfloat16
F8 = mybir.dt.float8e4
AF = mybir.ActivationFunctionType
OP = mybir.AluOpType
DR = mybir.MatmulPerfMode.DoubleRow

DIM, HEADS, WIN, MLP_H = 768, 12, 14, 3072
B, H0, W0 = 8, 64, 64
NTOK = H0 * W0
NW = 5            # windows per image axis (70/14)
NWIN = NW * NW    # 25 windows
WW = WIN * WIN    # 196 tokens per window
HC = 98           # half-window chunk (7 rows x 14 cols)
DH = DIM // HEADS # 64
EPS = 1e-5

# window pairing: 12 pairs + 1 single
PAIRS = [(2 * i, 2 * i + 1) for i in range(12)] + [(24,)]


def _chunk_geom(w, c):
    """Valid-row/col geometry of half-chunk c (0/1) of window w."""
    wi, wj = divmod(w, NW)
    r0 = wi * WIN + c * 7          # first padded-image row of this chunk
    c0 = wj * WIN
    vr = 7 if (wi < 4 or c == 0) else 1   # wi==4 -> rows 56..63 valid (8)
    vc = 14 if wj < 4 else 8
    return r0, c0, vr, vc


def _gather_chunk(nc, dst, dram, w, c, eng=None):
    """DMA image tokens of half-chunk (w, c) from [4096,768] DRAM into
    dst [98, 768] SBUF tile (partition p = 14*row + col). Pads with zeros."""
    eng = eng or nc.sync
    r0, c0, vr, vc = _chunk_geom(w, c)
    if vr < 7 or vc < 14:
        nc.gpsimd.memset(dst[:, :], 0.0)
    if vc == 14:
        src = bass.AP(tensor=dram, offset=(r0 * W0 + c0) * DIM,
                      ap=[[W0 * DIM, vr], [DIM, 14], [1, DIM]])
        eng.dma_start(dst[0:vr * 14, :], src)
    else:
        for r in range(vr):
            src = bass.AP(tensor=dram, offset=((r0 + r) * W0 + c0) * DIM,
                          ap=[[DIM, vc], [1, DIM]])
            eng.dma_start(dst[r * 14:r * 14 + vc, :], src)


def _scatter_chunk(nc, dram, src, w, c, eng=None):
    """DMA the valid tokens of half-chunk (w, c) from src [98,768] SBUF back
    to token-major [4096,768] DRAM."""
    eng = eng or nc.sync
    r0, c0, vr, vc = _chunk_geom(w, c)
    if vc == 14:
        dst = bass.AP(tensor=dram, offset=(r0 * W0 + c0) * DIM,
                      ap=[[W0 * DIM, vr], [DIM, 14], [1, DIM]])
        eng.dma_start(dst, src[0:vr * 14, :])
    else:
        for r in range(vr):
            dst = bass.AP(tensor=dram, offset=((r0 + r) * W0 + c0) * DIM,
                          ap=[[DIM, vc], [1, DIM]])
            eng.dma_start(dst, src[r * 14:r * 14 + vc, :])


def build_program():
    nc = bacc.Bacc(None, target_bir_lowering=False, debug=False)

    x_d = nc.dram_tensor("x", [NTOK, DIM], BF16, kind="ExternalInput")
    qkvw_d = nc.dram_tensor("qkvw", [DIM, 3 * DIM], F8, kind="ExternalInput")
    qkvb_d = nc.dram_tensor("qkvb", [2 * DIM], F32, kind="ExternalInput")
    projw_d = nc.dram_tensor("projw", [DIM, DIM], F8, kind="ExternalInput")
    projb_d = nc.dram_tensor("projb", [DIM], F32R, kind="ExternalInput")
    bt_d = nc.dram_tensor("bt", [HC, HEADS, 2, WW], BF16, kind="ExternalInput")
    sel_d = nc.dram_tensor("sel", [HEADS, 6 * 128], BF16, kind="ExternalInput")
    e16_d = nc.dram_tensor("e16", [128, 2 * HEADS * 128], F8, kind="ExternalInput")
    fc1w_d = nc.dram_tensor("fc1w", [DIM, MLP_H], F8, kind="ExternalInput")
    fc1b_d = nc.dram_tensor("fc1b", [MLP_H], F32, kind="ExternalInput")
    fc2w_d = nc.dram_tensor("fc2w", [MLP_H, DIM], F8, kind="ExternalInput")
    fc2b_d = nc.dram_tensor("fc2b", [DIM], F32R, kind="ExternalInput")

    out_d = nc.dram_tensor("out", [NTOK, DIM], F32, kind="ExternalOutput")
    x2_d = nc.dram_tensor("x2", [NTOK, DIM], BF16)  # internal scratch

    with tile.TileContext(nc) as tc:
        with ExitStack() as g:
            # ---------------- global constants / weights ----------------
            consts = g.enter_context(tc.tile_pool(name="consts", bufs=1))
            ident32 = consts.tile([128, 128], F32)
            make_identity(nc, ident32)
            identb = consts.tile([128, 128], BF16)
            nc.vector.tensor_copy(identb[:, :], ident32[:, :])
            ones32 = consts.tile([33, 128], F32)
            nc.vector.memset(ones32[:, :], 1.0)
            ones_r = consts.tile([33, 128], F32R)
            nc.vector.tensor_copy(ones_r[:, :], ones32[:, :])
            e16 = consts.tile([128, 2, HEADS, 128], F8)
            nc.gpsimd.dma_start(e16[:, :, :, :], e16_d[:, :]
                                .rearrange("p (a h m) -> p a h m", a=2, h=HEADS))
            eps_t = consts.tile([128, 1], F32)
            nc.vector.memset(eps_t[:, :], EPS)
            qkvb_sb = consts.tile([128, 12], F32)
            nc.sync.dma_start(
                qkvb_sb[:, :],
                bass.AP(tensor=qkvb_d, offset=0, ap=[[1, 128], [128, 12]]))
            bias2 = consts.tile([33, DIM], F32R)
            nc.sync.dma_start(bias2[0:1, :],
                              bass.AP(tensor=projb_d, offset=0, ap=[[1, DIM]]))
            nc.sync.dma_start(bias2[32:33, :],
                              bass.AP(tensor=fc2b_d, offset=0, ap=[[1, DIM]]))
            fc1b_sb = consts.tile([128, 24], F32)
            nc.sync.dma_start(
                fc1b_sb[:, :],
                bass.AP(tensor=fc1b_d, offset=0, ap=[[1, 128], [128, 24]]))
            sel_sb = consts.tile([HEADS, 6, 128], BF16)
            nc.scalar.dma_start(sel_sb[:, :, :], sel_d[:, :]
                                .rearrange("h (g p) -> h g p", p=128))
            bt_sb = consts.tile([HC, HEADS, 2, WW], BF16)
            for hh in range(3):
                eng = (nc.sync, nc.scalar, nc.gpsimd)[hh]
                eng.dma_start(bt_sb[:, 4 * hh:4 * (hh + 1), :, :],
                              bt_d[:, 4 * hh:4 * (hh + 1), :, :])
            stats2M = consts.tile([128, 32], F32)
            stats2R = consts.tile([128, 32], F32)

            # attention weights (fp8)
            wA = g.enter_context(tc.tile_pool(name="wA", bufs=1))
            qkvw_sb = wA.tile([128, 6, 3 * DIM], F8)
            for kk in range(6):
                for hh in range(2):
                    eng = (nc.sync, nc.scalar, nc.gpsimd)[(2 * kk + hh) % 3]
                    eng.dma_start(
                        qkvw_sb[:, kk, hh * 1152:(hh + 1) * 1152],
                        qkvw_d[kk * 128:(kk + 1) * 128,
                               hh * 1152:(hh + 1) * 1152])
            projw_sb = wA.tile([128, 6, DIM], F8)
            for kk in range(3):
                eng = (nc.sync, nc.scalar, nc.gpsimd)[kk]
                eng.dma_start(
                    projw_sb[:, 2 * kk:2 * kk + 2, :],
                    projw_d[kk * 256:(kk + 1) * 256, :]
                    .rearrange("(a p) n -> p a n", p=128))

            # MLP weights (fp8): tiles allocated now, DMAs emitted
            # after pair 0 so they don't delay the first gathers
            wB = g.enter_context(tc.tile_pool(name="wB", bufs=1))
            fc1w_sb = wB.tile([128, 6, MLP_H], F8)
            fc2w_sb = wB.tile([128, 24, DIM], F8)

            def load_fc_weights():
                for kk in range(6):
                    eng = (nc.sync, nc.scalar, nc.gpsimd)[kk % 3]
                    eng.dma_start(fc1w_sb[:, kk, :],
                                  fc1w_d[kk * 128:(kk + 1) * 128, :])
                for kk in range(8):
                    eng = (nc.sync, nc.scalar, nc.gpsimd)[kk % 3]
                    eng.dma_start(
                        fc2w_sb[:, 3 * kk:3 * (kk + 1), :],
                        fc2w_d[kk * 384:(kk + 1) * 384, :]
                        .rearrange("(a p) n -> p a n", p=128))

            # natural_log_exp_and_others: exp (softmax) + ln/exp (rstd) + gelu
            nc.scalar.add_instruction(mybir.InstLoadActFuncSet(
                name=nc.get_next_instruction_name(), ins=[], outs=[],
                act_func_set_id=6))

            # ---------------- attention over window pairs ----------------
            with ExitStack() as s2:
                pxp = s2.enter_context(tc.tile_pool(name="pxp", bufs=2))
                pln = s2.enter_context(tc.tile_pool(name="pln", bufs=2))
                phT = s2.enter_context(tc.tile_pool(name="phT", bufs=2))
                pqk = s2.enter_context(tc.tile_pool(name="pqk", bufs=2))
                pvT = s2.enter_context(tc.tile_pool(name="pvT", bufs=2))
                vt_alloc = [0]
                pUT = s2.enter_context(tc.tile_pool(name="pUT", bufs=3))
                ut_alloc = [0]
                prd = s2.enter_context(tc.tile_pool(name="prd", bufs=2))
                prr = s2.enter_context(tc.tile_pool(name="prr", bufs=3))
                pOT = s2.enter_context(tc.tile_pool(name="pOT", bufs=2))
                px2 = s2.enter_context(tc.tile_pool(name="px2", bufs=3))
                pstat = s2.enter_context(tc.tile_pool(name="pstat", bufs=3))
                # MLP pools (super-tiles are pumped into the pair loop)
                pxt = s2.enter_context(tc.tile_pool(name="pxt", bufs=7))
                pxn = s2.enter_context(tc.tile_pool(name="pxn", bufs=3))
                pxnx = s2.enter_context(tc.tile_pool(name="pxnx", bufs=5))
                pxnT = s2.enter_context(tc.tile_pool(name="pxnT", bufs=2))
                pgT = s2.enter_context(tc.tile_pool(name="pgT", bufs=2))
                pout = s2.enter_context(tc.tile_pool(name="pout", bufs=2))
                pst = s2.enter_context(tc.tile_pool(name="pst", bufs=2,
                                                   space="PSUM"))
                pps = s2.enter_context(tc.tile_pool(name="pps", bufs=5,
                                                   space="PSUM"))
                psd = s2.enter_context(tc.tile_pool(name="psd", bufs=1,
                                                   space="PSUM"))

                def prep_pair_units(pair, out_holder):
                    """gather + LN1 for a pair as a 4-unit generator, pumped
                    from inside the PREVIOUS pair's stage 1 so the DVE stats
                    work overlaps the ACT exp bursts instead of forming an
                    ACT/DVE convoy. rstd comes from a 4-chunk-batched DVE
                    Newton rsqrt: no ACT table funcs here, so pumped gelus
                    don't thrash the activation table."""
                    x_pair = pxp.tile([HC, 4, DIM], BF16, name="x_pair")
                    ln_pair = pln.tile([HC, 4, DIM], BF16, name="ln_pair")
                    out_holder.append((x_pair, ln_pair))
                    nch_p = 2 * len(pair)
                    mvp = pstat.tile([HC, 4, 2], F32, tag="mv", bufs=2,
                                     name="mvp")
                    for wl, w in enumerate(pair):
                        for c in range(2):
                            i = 2 * wl + c
                            _gather_chunk(nc, x_pair[:, i, :], x_d, w, c,
                                          eng=nc.gpsimd)
                            st = pstat.tile([HC, 3, 6], F32, name="bst")
                            for gg in range(3):
                                nc.vector.bn_stats(
                                    st[:, gg, :],
                                    x_pair[:, i, gg * 256:(gg + 1) * 256])
                            nc.vector.bn_aggr(mvp[:, i, :], st[:, :, :])
                            if i < nch_p - 1:
                                yield
                    # batched rsqrt(var+eps) for all chunks (quake + Newton)
                    nw8 = pstat.tile([HC, 6, 4], F32, tag="nw", bufs=2,
                                     name="nw8")
                    ve = nw8[:, 0, :nch_p]
                    nc.vector.tensor_scalar(
                        out=ve, in0=mvp[:, :nch_p, 1], scalar1=EPS,
                        scalar2=None, op0=OP.add)
                    yi = nw8[:, 1, :nch_p].bitcast(mybir.dt.int32)
                    nc.vector.tensor_scalar(
                        out=yi, in0=ve.bitcast(mybir.dt.int32),
                        scalar1=1, scalar2=None, op0=OP.arith_shift_right)
                    y0 = nw8[:, 2, :nch_p].bitcast(mybir.dt.int32)
                    nc.vector.tensor_scalar(
                        out=y0, in0=yi, scalar1=-1,
                        scalar2=None, op0=OP.bitwise_xor)
                    nc.vector.tensor_scalar(
                        out=y0, in0=y0, scalar1=0x5f3759e0,
                        scalar2=None, op0=OP.add)
                    ya, yb = nw8[:, 2, :nch_p], nw8[:, 3, :nch_p]
                    t2, w5 = nw8[:, 4, :nch_p], nw8[:, 5, :nch_p]
                    for _ in range(3):
                        nc.vector.tensor_tensor(
                            out=t2, in0=ya, in1=ya, op=OP.mult)
                        nc.vector.scalar_tensor_tensor(
                            out=w5, in0=ve, scalar=-0.5, in1=t2,
                            op0=OP.mult, op1=OP.mult)
                        nc.vector.tensor_scalar(
                            out=w5, in0=w5, scalar1=1.5,
                            scalar2=None, op0=OP.add)
                        nc.vector.tensor_tensor(
                            out=yb, in0=ya, in1=w5, op=OP.mult)
                        ya, yb = yb, ya
                    for i in range(nch_p):
                        nc.vector.tensor_scalar(
                            out=ln_pair[:, i, :], in0=x_pair[:, i, :],
                            scalar1=mvp[:, i, 0:1],
                            scalar2=ya[:, i:i + 1],
                            op0=OP.subtract, op1=OP.mult)

                def mlp_st_units(st):
                    """One 512-token MLP super-tile as a lazy unit stream
                    (16 yields): 4x chunk prep (DMA + LN2 stats, no PE),
                    4x transpose, 4x fc1 (6 m-blocks each), 4x fc2 chunk.
                    Units are pumped into the attention pair loop so the
                    in-order engine queues fill attention's bubbles; x2 rows
                    [512*st, 512*st+512) must already be scattered."""
                    x2nT = pxnT.tile([128, 6, 512], F8, name="x2nT")
                    x2t, xns = [], []
                    for c in range(4):
                        t = st * 4 + c
                        xt = pxt.tile([128, DIM], BF16, name="x2t")
                        # never on ACT/DVE: this load WAITS on the x2
                        # scatters, and a DMA's waits hold the issuing
                        # engine's queue
                        eng = nc.gpsimd if t % 2 == 0 else nc.sync
                        eng.dma_start(xt[:, :], x2_d[t * 128:(t + 1) * 128, :])
                        x2t.append(xt)
                        # LN2 stats inline; rsqrt on DVE only
                        # (quake seed + 3 Newton steps)
                        st5 = pxn.tile([128, 3, 6], F32, name="st5")
                        for gg in range(3):
                            nc.vector.bn_stats(
                                st5[:, gg, :],
                                xt[:, gg * 256:(gg + 1) * 256])
                        mv5 = pxn.tile([128, 8], F32, name="mv5")
                        nc.vector.bn_aggr(mv5[:, 0:2], st5[:, :, :])
                        nc.gpsimd.tensor_copy(stats2M[:, t % 32:t % 32 + 1],
                                              mv5[:, 0:1])
                        # LN2 stats inline; rsqrt on DVE only
                        # (quake seed + 3 Newton steps)
                        ve = mv5[:, 2:3]
                        nc.vector.tensor_scalar(
                            out=ve, in0=mv5[:, 1:2], scalar1=EPS,
                            scalar2=None, op0=OP.add)
                        yi = mv5[:, 3:4].bitcast(mybir.dt.int32)
                        nc.vector.tensor_scalar(
                            out=yi, in0=ve.bitcast(mybir.dt.int32),
                            scalar1=1, scalar2=None,
                            op0=OP.arith_shift_right)
                        y0 = mv5[:, 4:5].bitcast(mybir.dt.int32)
                        nc.vector.tensor_scalar(
                            out=y0, in0=yi, scalar1=-1,
                            scalar2=None, op0=OP.bitwise_xor)
                        nc.vector.tensor_scalar(
                            out=y0, in0=y0, scalar1=0x5f3759e0,
                            scalar2=None, op0=OP.add)
                        ya, yb = mv5[:, 4:5], mv5[:, 5:6]
                        t2, w5 = mv5[:, 6:7], mv5[:, 7:8]
                        for _ in range(3):
                            nc.vector.tensor_tensor(
                                out=t2, in0=ya, in1=ya, op=OP.mult)
                            nc.vector.scalar_tensor_tensor(
                                out=w5, in0=ve, scalar=-0.5, in1=t2,
                                op0=OP.mult, op1=OP.mult)
                            nc.vector.tensor_scalar(
                                out=w5, in0=w5, scalar1=1.5,
                                scalar2=None, op0=OP.add)
                            nc.vector.tensor_tensor(
                                out=yb, in0=ya, in1=w5, op=OP.mult)
                            ya, yb = yb, ya
                        nc.gpsimd.tensor_copy(stats2R[:, t % 32:t % 32 + 1], ya)
                        xn = pxnx.tile([128, DIM], BF16, name="x2n")
                        nc.vector.tensor_scalar(
                            out=xn[:, :], in0=xt[:, :],
                            scalar1=stats2M[:, t % 32:t % 32 + 1],
                            scalar2=stats2R[:, t % 32:t % 32 + 1],
                            op0=OP.subtract, op1=OP.mult)
                        xns.append(xn)
                        yield 'f'
                    for c in range(4):
                        for k2 in range(3):
                            ps_t2 = pps.tile([128, 2, 128], BF16, tag="ps",
                                             name="ps_t2")
                            for kk2 in range(2):
                                k = 2 * k2 + kk2
                                nc.tensor.transpose(
                                    ps_t2[:, kk2, :],
                                    xns[c][:, k * 128:(k + 1) * 128],
                                    identb[:, :])
                            # x4: fp8 range scaling for fc1 (w also x4)
                            nc.vector.tensor_scalar(
                                out=x2nT[:, 2 * k2:2 * k2 + 2,
                                         c * 128:(c + 1) * 128],
                                in0=ps_t2[:, :, :],
                                scalar1=4.0, scalar2=None, op0=OP.mult)
                        yield ('g' if c == 3 else 'f')
                    gT = pgT.tile([128, 24, 512], F8, name="gT")
                    for mg in range(4):
                        for m in range(6 * mg, 6 * mg + 6):
                            ps_f1 = pst.tile([128, 512], F32, tag="st",
                                             name="ps_f1")
                            for kk in range(3):
                                nc.tensor.matmul(
                                    ps_f1[:, :],
                                    fc1w_sb[:, 2 * kk:2 * kk + 2,
                                            m * 128:(m + 1) * 128],
                                    x2nT[:, 2 * kk:2 * kk + 2, :],
                                    start=(kk == 0), stop=(kk == 2),
                                    perf_mode=DR)
                            # psum = 16 * preact; gelu(ps/16 + b)
                            nc.scalar.activation(
                                gT[:, m, :], ps_f1[:, :], AF.Gelu,
                                bias=fc1b_sb[:, m:m + 1], scale=1.0 / 16.0)
                        yield ('g' if mg < 3 else 'f')
                    for c in range(4):
                        outt = pout.tile([128, DIM], F32, name="outt")
                        for n in range(2):
                            ps_f2 = pps.tile([128, 384], F32, tag="ps",
                                             name="ps_f2")
                            for m in range(12):
                                nc.tensor.matmul(
                                    ps_f2[:, :384],
                                    gT[:, 2 * m:2 * m + 2,
                                       c * 128:(c + 1) * 128],
                                    fc2w_sb[:, 2 * m:2 * m + 2,
                                            n * 384:(n + 1) * 384],
                                    start=(m == 0), stop=False,
                                    perf_mode=DR)
                            nc.tensor.matmul(
                                ps_f2[:, :384],
                                ones_r[32:33, :128],
                                bias2[32:33, n * 384:(n + 1) * 384],
                                start=False, stop=True)
                            nc.vector.scalar_tensor_tensor(
                                out=outt[:, n * 384:(n + 1) * 384],
                                in0=ps_f2[:, :384],
                                scalar=1.0 / 16.0,
                                in1=x2t[c][:, n * 384:(n + 1) * 384],
                                op0=OP.mult, op1=OP.add)
                        t = st * 4 + c
                        nc.gpsimd.dma_start(
                            out_d[t * 128:(t + 1) * 128, :], outt[:, :])
                        yield 'f'

                # super-tile st readable once the pair covering its last
                # image row has scattered: rows 8st..8st+7 -> window row
                # (8st+7)//14 -> last window 5*wrow+4 -> pair (5*wrow+4)//2
                UNLOCK = {2: [0], 4: [1, 2], 7: [3, 4], 9: [5, 6], 12: [7]}
                mlp_q = []   # entries [generator, kind-of-next-unit]

                def pump(n, want=None):
                    while mlp_q and n > 0:
                        g, kind = mlp_q[0]
                        if want is not None and kind != want:
                            return
                        try:
                            mlp_q[0][1] = next(g)
                            n -= 1
                        except StopIteration:
                            mlp_q.pop(0)

                prepq = []
                prep_state = [None]

                def pump_prep(n=1):
                    while prep_state[0] is not None and n > 0:
                        try:
                            next(prep_state[0])
                        except StopIteration:
                            prep_state[0] = None
                        n -= 1

                def start_prep(pair):
                    prep_state[0] = prep_pair_units(pair, prepq)
                    pump_prep(1)

                start_prep(PAIRS[0])
                pump_prep(8)
                for pi, pair in enumerate(PAIRS):
                    nw = len(pair)
                    PW = nw * WW          # 392 or 196
                    nch = 2 * nw          # chunks in pair
                    x_pair, ln_pair = prepq.pop(0)

                    # transpose ln (bf16) -> evac casts to fp8 hT [128, 6, PW]
                    # (padded to 512 cols: vT stationary slabs read 128-wide
                    # at 98-strides; tail is zeroed, extra rows discarded)
                    hT = phT.tile([128, 6, 512], F8, name="hT")
                    nc.gpsimd.memset(hT[:, :, PW:], 0.0)
                    for k in range(6):
                        ps_t = pps.tile([128, 392], BF16, tag="ps", name="ps_t")
                        for i in range(nch):
                            nc.tensor.transpose(
                                ps_t[:, i * HC:(i + 1) * HC],
                                ln_pair[:HC, i, k * 128:(k + 1) * 128],
                                identb[:HC, :HC])
                        nc.vector.tensor_copy(hT[:, k, :PW], ps_t[:, :PW])
                    pump(1)

                    # fused q,k features [1536, PW] via DoubleRow fp8.
                    # Emit interleaved (0,6,1,7,..) so head h's operands
                    # finish earliest.
                    qkT = pqk.tile([128, 12, 2 * WW], F8, name="qkT")
                    for m in [0, 6, 1, 7, 2, 8, 3, 9, 4, 10, 5, 11]:
                        ps_qk = pps.tile([128, 392], F32, tag="ps", name="ps_qk")
                        for kk in range(3):
                            nc.tensor.matmul(
                                ps_qk[:, :PW],
                                qkvw_sb[:, 2 * kk:2 * kk + 2,
                                        m * 128:(m + 1) * 128],
                                hT[:, 2 * kk:2 * kk + 2, :PW],
                                start=(kk == 0), stop=(kk == 2),
                                perf_mode=DR)
                        if m % 3 == 2:
                            nc.vector.tensor_scalar(
                                out=qkT[:, m, :PW], in0=ps_qk[:, :PW],
                                scalar1=qkvb_sb[:, m:m + 1], scalar2=None,
                                op0=OP.add)
                        else:
                            nc.scalar.activation(
                                qkT[:, m, :PW], ps_qk[:, :PW],
                                AF.Identity, bias=qkvb_sb[:, m:m + 1])
                    pump(2)

                    # v token-major fp8, checkerboard layout [128, chunk, 2
                    # K-slabs, 768]: even heads' 64-feature blocks in slab 0,
                    # odd heads' in slab 1; complementary halves and rows
                    # 98:128 stay zero (memset once per ring buffer). AV then
                    # contracts a head-pair in DoubleRow matmuls whose
                    # 128-row outputs start at partition 0 (ISA quadrant rule
                    # forbids DR dst at partition 64).
                    vT = pvT.tile([128, 4, 2, DIM], F8, name="vT")
                    if vt_alloc[0] < 2:
                        nc.gpsimd.memset(vT[:, :, :, :], 0.0)
                        vt_alloc[0] += 1
                    for i in range(nch):
                        for n in range(2):
                            ps_v = pps.tile([128, 384], F32, tag="ps", name="ps_v")
                            for kk in range(3):
                                nc.tensor.matmul(
                                    ps_v[:, :384],
                                    hT[:, 2 * kk:2 * kk + 2, i * HC:i * HC + 128],
                                    qkvw_sb[:, 2 * kk:2 * kk + 2,
                                            2 * DIM + n * 384:2 * DIM + (n + 1) * 384],
                                    start=(kk == 0), stop=(kk == 2),
                                    perf_mode=DR)
                            # col c -> slab (c//64)%2, feature col c
                            dst = bass.AP(
                                tensor=vT.tensor,
                                offset=i * 2 * DIM + n * 384,
                                ap=[[vT.ap[0][0], HC],
                                    [128, 3], [DIM + 64, 2], [1, 64]])
                            if n == 0:
                                nc.scalar.copy(dst, ps_v[:HC, :384])
                            else:
                                nc.vector.tensor_copy(dst, ps_v[:HC, :384])

                    if pi + 1 < len(PAIRS):
                        start_prep(PAIRS[pi + 1])
                    if pi == 0:
                        load_fc_weights()
                    pump(2)

                    OT = pOT.tile([128, 6, 512], F8, name="OT")
                    nc.gpsimd.memset(OT[:, :, PW:], 0.0)

                    # stage 1 (window-outer so only one den group is pending
                    # per bank): S^T + 128*B -> exp -> U^T fp8 -> den.
                    # U^T for all heads of a window lives in one tile
                    # [128, 12, 2, WW] so AV can slice two adjacent heads as
                    # DoubleRow K-slabs.
                    rden = prd.tile([HEADS, 2 * WW], BF16, name="rden")
                    UTw = [None] * nw
                    for wl in range(nw):
                        ps_den = psd.tile([128, 512], F32, tag="psd",
                                          name="ps_den")
                        UT = pUT.tile([128, HEADS, 2, WW], F8, name="UT")
                        UTw[wl] = UT
                        if ut_alloc[0] < 3:
                            # whole tile zeroed once per ring buffer: rows
                            # 98:128 must stay zero for the K=128 DoubleRow
                            # den/AV, and pad-q columns of edge windows are
                            # never written by exp (skipped), so they must
                            # start finite
                            nc.gpsimd.memset(UT[:, :, :, :], 0.0)
                            ut_alloc[0] += 1

                        def emit_den(h):
                            # den (x16): K=128/M=128 DoubleRow over both kc
                            # chunks at once (rows >= 98 of e16/UT are zero).
                            # skip_group_check: the group legitimately stays
                            # open across all heads; the interp's conservative
                            # region marking false-positives on other banks.
                            nc.tensor.matmul(
                                ps_den[:, :WW],
                                e16[:, :, h, :],
                                UT[:, h, :, :],
                                start=(h == 0), stop=(h == HEADS - 1),
                                perf_mode=DR,
                                skip_group_check=True)

                        # pad-query skip for edge windows: scores/bias only
                        # stream the valid q columns (packed in ps_S), exp
                        # scatters them back to window-coord UT positions.
                        # Pad-q lanes of UT/den/OT carry garbage that only
                        # ever reaches unscattered pad-token rows.
                        w_abs = pair[wl]
                        wwi, wwj = divmod(w_abs, NW)

                        def q_in(ap196):
                            """valid-q view of an AP whose last dim is WW"""
                            if wwj == 4:
                                v = ap196.rearrange("p (r c) -> p r c", c=WIN)
                                return (v[:, 0:8, 0:8] if wwi == 4
                                        else v[:, :, 0:8])
                            if wwi == 4:
                                return ap196[:, 0:112]
                            return ap196

                        NQ = (64 if (wwi == 4 and wwj == 4) else
                              112 if (wwi == 4 or wwj == 4) else WW)
                        packed = wwj == 4   # valid q non-contiguous: pack

                        prev = None
                        for h in range(HEADS):
                            qrow = (h % 2) * 64
                            qm = h // 2
                            km = 6 + h // 2
                            ps_S = pst.tile([128, 512], F32, tag="st",
                                            name="ps_S")
                            for kc in range(2):
                                if packed:
                                    s_out = ps_S[:HC, kc * NQ:(kc + 1) * NQ]
                                else:
                                    s_out = ps_S[:HC, kc * WW:kc * WW + NQ]
                                nc.tensor.matmul(
                                    s_out,
                                    qkT[qrow:qrow + 64, km,
                                        (2 * wl + kc) * HC:(2 * wl + kc + 1) * HC],
                                    q_in(qkT[qrow:qrow + 64, qm,
                                             wl * WW:(wl + 1) * WW]),
                                    start=True, stop=False)
                                nc.tensor.matmul(
                                    s_out,
                                    identb[:HC, :HC],
                                    q_in(bt_sb[:, h, kc, :]),
                                    start=False, stop=True)
                            # den of head h-1 here: keeps the PE stream from
                            # stalling on exp(h) before scores(h+1)
                            if prev is not None:
                                emit_den(prev)
                            if packed:
                                e_in = (ps_S[:HC, 0:2 * NQ]
                                        .rearrange("p (k r c) -> p k r c",
                                                   k=2, c=8))
                                ut_slice = UT[:HC, h, :, :].rearrange(
                                    "p k (r c) -> p k r c", c=WIN)
                                e_out = (ut_slice[:, :, 0:8, 0:8]
                                         if wwi == 4
                                         else ut_slice[:, :, :, 0:8])
                            elif NQ < WW:
                                e_in = (ps_S[:HC, 0:2 * WW]
                                        .rearrange("p (k q) -> p k q", k=2)
                                        [:, :, 0:NQ])
                                e_out = UT[:HC, h, :, 0:NQ]
                            else:
                                e_in = ps_S[:HC, 0:2 * WW]
                                e_out = UT[:HC, h, :, :]
                            nc.scalar.activation(
                                e_out, e_in,
                                AF.Exp, scale=1.0 / 128.0)
                            prev = h
                            if h in (2, 5, 8):
                                pump(2, want='f')
                            elif h == 9:
                                pump_prep(1)
                        emit_den(prev)
                        with nc.allow_low_precision("1/den in bf16: 0.4% on "
                                                    "a normalizer, under fp8 "
                                                    "operand noise"):
                            nc.vector.reciprocal(
                                rden[:, wl * WW:(wl + 1) * WW],
                                ps_den[:HEADS, :WW])
                        pump(1, want='f')
                        pump_prep(1)

                    # stage 2, fused per head-pair j = (2j, 2j+1): both
                    # heads land in one [128, PW] PSUM tile (rows 0-63 /
                    # 64-127) via checkerboard-vT DoubleRow matmuls,
                    # normalized by one combined rdrep in one DVE op.
                    def pair_av(j, rdrep):
                        ps_O = pps.tile([128, 392], F32, tag="ps", name="ps_O")
                        for wl in range(nw):
                            for kc in range(2):
                                nc.tensor.matmul(
                                    ps_O[:, wl * WW:(wl + 1) * WW],
                                    vT[:, 2 * wl + kc, :,
                                       j * 128:(j + 1) * 128],
                                    UTw[wl][:, 2 * j:2 * j + 2, kc, :],
                                    start=(kc == 0), stop=(kc == 1),
                                    perf_mode=DR)
                        nc.vector.tensor_tensor(
                            out=OT[:, j, :PW],
                            in0=ps_O[:, :PW],
                            in1=rdrep[:, :PW],
                            op=OP.mult)

                    # rep(j+1) emitted before AV(j): PE never stalls on the
                    # ACT rdrep copy
                    def emit_rep(j):
                        ps_rep = pps.tile([128, 392], F32, tag="ps",
                                          name="ps_rep")
                        nc.tensor.matmul(
                            ps_rep[:, :PW],
                            sel_sb[:, j, :],
                            rden[:, :PW],
                            start=True, stop=True)
                        rdrep = prr.tile([128, 2 * WW], BF16, name="rdrep")
                        nc.scalar.copy(rdrep[:, :PW], ps_rep[:, :PW])
                        return rdrep

                    rdrep_cur = emit_rep(0)
                    for j in range(6):
                        rdrep_nxt = emit_rep(j + 1) if j + 1 < 6 else None
                        pair_av(j, rdrep_cur)
                        rdrep_cur = rdrep_nxt
                    pump(2)

                    # proj (DR fp8) + 1/16 + residual -> x2 bf16, scatter
                    for i in range(nch):
                        x2c = px2.tile([HC, DIM], BF16, name="x2c")
                        for n in range(2):
                            ps_pr = pps.tile([128, 384], F32, tag="ps", name="ps_pr")
                            for kk in range(3):
                                nc.tensor.matmul(
                                    ps_pr[:, :384],
                                    OT[:, 2 * kk:2 * kk + 2, i * HC:i * HC + 128],
                                    projw_sb[:, 2 * kk:2 * kk + 2,
                                             n * 384:(n + 1) * 384],
                                    start=(kk == 0), stop=False,
                                    perf_mode=DR)
                            nc.tensor.matmul(
                                ps_pr[:, :384],
                                ones_r[0:1, :128],
                                bias2[0:1, n * 384:(n + 1) * 384],
                                start=False, stop=True)
                            nc.vector.scalar_tensor_tensor(
                                out=x2c[:, n * 384:(n + 1) * 384],
                                in0=ps_pr[:HC, :384],
                                scalar=1.0 / 16.0,
                                in1=x_pair[:, i, n * 384:(n + 1) * 384],
                                op0=OP.mult, op1=OP.add)
                        _scatter_chunk(nc, x2_d, x2c[:, :], pair[i // 2], i % 2,
                                       eng=nc.sync)
                    pump_prep(8)
                    pump(2)

                    for st in UNLOCK.get(pi, []):
                        mlp_q.append([mlp_st_units(st), 'f'])

                # drain remaining MLP units
                pump(1 << 30)

    nc.compile()
    return nc


_NC_CACHE = {}


def _get_nc():
    if "nc" not in _NC_CACHE:
        _NC_CACHE["nc"] = build_program()
    return _NC_CACHE["nc"]


def _prep_weights(inputs):
    f = lambda k: np.asarray(inputs[k], np.float32)
    x = f("x")
    ln1_g, ln1_b = f("ln1_g"), f("ln1_b")
    ln2_g, ln2_b = f("ln2_g"), f("ln2_b")
    qkv_w, qkv_b = f("qkv_w"), f("qkv_b")
    proj_w, proj_b = f("proj_w"), f("proj_b")
    fc1_w, fc1_b = f("fc1_w"), f("fc1_b")
    fc2_w, fc2_b = f("fc2_w"), f("fc2_b")
    rel = f("rel_pos_bias")
    SCALE = np.float32((DIM // HEADS) ** -0.5)

    # fold ln1 into qkv
    qkv_w_f = qkv_w * ln1_g[None, :]
    qkv_b_f = qkv_w @ ln1_b + qkv_b
    # fold v bias into proj bias (softmax rows sum to 1)
    projb = proj_b + proj_w @ qkv_b_f[2 * DIM:]
    # fp8 scaling: q rows *(8*SCALE)=1.0, k rows *16, v rows *16
    qkv_w_s = qkv_w_f.copy()
    qkv_w_s[:DIM] *= 8.0 * SCALE
    qkv_w_s[DIM:] *= 16.0
    qkv_b_s = qkv_b_f[:2 * DIM].copy()
    qkv_b_s[:DIM] *= 8.0 * SCALE
    qkv_b_s[DIM:] *= 16.0
    # proj: *16 (OT is true-scale); output scaled back by 1/16 in DVE
    projw_s = proj_w * 16.0
    projb_s = projb * 16.0
    # fold ln2 into fc1
    fc1_w_f = fc1_w * ln2_g[None, :]
    fc1_b_f = fc1_w @ ln2_b + fc1_b

    # 128*B transposed: bt[p, h, kc, q] = 128*rel[h, q, kc*98+p]
    bt = (128.0 * rel).transpose(0, 2, 1)          # [h, k, q]
    bt = bt.reshape(HEADS, 2, HC, WW).transpose(2, 0, 1, 3).copy()

    # sel[h', j, p] = 1 where h' = 2j (p<64) or 2j+1 (p>=64): one
    # replicate-matmul builds both heads' 1/den rows of an OT m-chunk
    sel = np.zeros((HEADS, 6, 128), np.float32)
    for j in range(6):
        sel[2 * j, j, :64] = 1.0
        sel[2 * j + 1, j, 64:] = 1.0
    sel = sel.reshape(HEADS, 6 * 128)

    e16 = np.zeros((128, 2, HEADS, 128), np.float32)
    for h in range(HEADS):
        e16[:HC, :, h, h] = 16.0
    e16 = e16.reshape(128, 2 * HEADS * 128)

    e4 = ml_dtypes.float8_e4m3
    b16 = ml_dtypes.bfloat16
    return {
        "qkvw": np.ascontiguousarray(qkv_w_s.T).astype(e4),   # [768, 2304]
        "qkvb": qkv_b_s,
        "projw": np.ascontiguousarray(projw_s.T).astype(e4),  # [768, 768]
        "projb": projb_s,
        "bt": bt.astype(b16),
        "sel": sel.astype(b16),
        "e16": e16.astype(e4),
        "fc1w": np.ascontiguousarray(fc1_w_f.T * 4.0).astype(e4),  # [768, 3072]
        "fc1b": fc1_b_f,
        "fc2w": np.ascontiguousarray(fc2_w.T * 16.0).astype(e4),  # [3072, 768]
        "fc2b": fc2_b * 16.0,
    }, x


PROFILE = False
LAST_RESULT = None


def prepare_exec(inputs):
    """bench.py hook: (nc, per-core in_maps, n_cores)."""
    weights, x = _prep_weights(inputs)
    nc = _get_nc()
    in_maps = [dict(weights, x=np.ascontiguousarray(x[i]).astype(ml_dtypes.bfloat16))
               for i in range(B)]
    return nc, in_maps, B


def kernel(**inputs):
    global LAST_RESULT
    weights, x = _prep_weights(inputs)
    nc = _get_nc()
    in_maps = [dict(weights, x=np.ascontiguousarray(x[i]).astype(ml_dtypes.bfloat16))
               for i in range(B)]
    res = run_bass_kernel_spmd(nc, in_maps, core_ids=list(range(B)),
                               trace=PROFILE)
    LAST_RESULT = res
    out = np.stack([res.results[i]["out"] for i in range(B)], axis=0)
    return out.astype(np.float32)


if __name__ == "__main__":
    rng = np.random.default_rng(0)
    ins = {
        "x": rng.standard_normal((B, NTOK, DIM), dtype=np.float32),
        "rel_pos_bias": (rng.standard_normal((HEADS, WW, WW)) * 0.1).astype(np.float32),
        "ln1_g": np.ones(DIM, np.float32), "ln1_b": np.zeros(DIM, np.float32),
        "qkv_w": (rng.standard_normal((3 * DIM, DIM)) * 0.02).astype(np.float32),
        "qkv_b": np.zeros(3 * DIM, np.float32),
        "proj_w": (rng.standard_normal((DIM, DIM)) * 0.02).astype(np.float32),
        "proj_b": np.zeros(DIM, np.float32),
        "ln2_g": np.ones(DIM, np.float32), "ln2_b": np.zeros(DIM, np.float32),
        "fc1_w": (rng.standard_normal((MLP_H, DIM)) * 0.02).astype(np.float32),
        "fc1_b": np.zeros(MLP_H, np.float32),
        "fc2_w": (rng.standard_normal((DIM, MLP_H)) * 0.02).astype(np.float32),
        "fc2_b": np.zeros(DIM, np.float32),
        "H": np.int64(64), "W": np.int64(64),
    }
    out = kernel(**ins)
    print("out", out.shape, out.dtype, np.abs(out).max())



# revision 73
# speedup vs baseline: 2.5802x; 2.5802x over previous
"""Swin-style transformer block on 8 Trainium2 NeuronCores.

Sharding: data-parallel over batch - each of the 8 cores processes one image
([4096, 768] tokens). All weights replicated per core. No collectives.

v3 design (vs v2):
  - Attention GEMMs in fp8e4m3 with DoubleRow (qkv, proj, AV, den), scores in
    fp8 at 1 cycle/row with window-local moving dims (no cross-window waste).
  - Scores computed TRANSPOSED (S^T[k,q] per head/window): exp writes U^T fp8
    directly; rel-pos bias is added into the scores PSUM by a PE
    identity-matmul (128*B, exp scale=1/128). Edge windows (wi==4 / wj==4)
    stream only their valid query columns (packed in PSUM, exp scatters back
    to window coords); garbage in pad-q lanes only ever reaches unscattered
    pad-token rows.
  - AV contracts both kc chunks of a head-pair in K=256 DoubleRow matmuls
    against a checkerboard vT layout (even heads slab 0 / odd heads slab 1,
    complementary halves zero), 128-row outputs at partition 0 (ISA rule).
  - Softmax denominator: per-head K=128/M=128 DoubleRow selector-matmuls
    accumulate all heads into one PSUM tile per window; one DVE reciprocal;
    per head-pair a selector-matmul replicates 1/den, applied on the OT
    evacuation in one DVE multiply.
  - MLP entirely fp8 DoubleRow (fc1 x4/x4 scaling, gelu(ps/16+b) on evac;
    fc2 x16 with 1/16 folded into the residual DVE op), 512-token
    super-tiles, full 3072 hidden resident in SBUF.
  - MLP super-tiles are emitted as fine-grained generator units PUMPED into
    the attention pair loop (in-order engine queues then fill attention's
    exp/dependency bubbles with MLP work). Units carrying gelus are only
    pumped at points away from exp bursts (exp and gelu live in different
    ACT tables; 1283ns load per switch). LN1 prep for the next pair is
    likewise a generator pumped from inside stage 1 (DVE stats overlap ACT
    exp bursts). x2/out DMAs ride queues whose waits can't block compute
    (a DMA's dependency waits hold the issuing engine's queue).
  - LN rstd via batched DVE quake+Newton rsqrt (no Ln/Exp table funcs).
    x is fed from host as bf16 (halves gather bytes; residual quantization
    is equivalent to the existing bf16 x2 roundtrip).
"""

import numpy as np
from contextlib import ExitStack

import ml_dtypes

import concourse.bass as bass
import concourse.mybir as mybir
import concourse.tile as tile
from concourse import bacc
from concourse.bass_utils import run_bass_kernel_spmd
from concourse.masks import make_identity

F32 = mybir.dt.float32
F32R = mybir.dt.float32r
BF16 = mybir.dt.bfloat16
F8 = mybir.dt.float8e4
AF = mybir.ActivationFunctionType
OP = mybir.AluOpType
DR = mybir.MatmulPerfMode.DoubleRow

DIM, HEADS, WIN, MLP_H = 768, 12, 14, 3072
B, H0, W0 = 8, 64, 64
NTOK = H0 * W0
NW = 5            # windows per image axis (70/14)
NWIN = NW * NW    # 25 windows
WW = WIN * WIN    # 196 tokens per window
HC = 98           # half-window chunk (7 rows x 14 cols)
DH = DIM // HEADS # 64
EPS = 1e-5

# window pairing: 12 pairs + 1 single
PAIRS = [(2 * i, 2 * i + 1) for i in range(12)] + [(24,)]


def _chunk_geom(w, c):
    """Valid-row/col geometry of half-chunk c (0/1) of window w."""
    wi, wj = divmod(w, NW)
    r0 = wi * WIN + c * 7          # first padded-image row of this chunk
    c0 = wj * WIN
    vr = 7 if (wi < 4 or c == 0) else 1   # wi==4 -> rows 56..63 valid (8)
    vc = 14 if wj < 4 else 8
    return r0, c0, vr, vc


def _gather_chunk(nc, dst, dram, w, c, eng=None):
    """DMA image tokens of half-chunk (w, c) from [4096,768] DRAM into
    dst [98, 768] SBUF tile (partition p = 14*row + col). Pads with zeros."""
    eng = eng or nc.sync
    r0, c0, vr, vc = _chunk_geom(w, c)
    if vr < 7 or vc < 14:
        nc.gpsimd.memset(dst[:, :], 0.0)
    if vc == 14:
        src = bass.AP(tensor=dram, offset=(r0 * W0 + c0) * DIM,
                      ap=[[W0 * DIM, vr], [DIM, 14], [1, DIM]])
        eng.dma_start(dst[0:vr * 14, :], src)
    else:
        for r in range(vr):
            src = bass.AP(tensor=dram, offset=((r0 + r) * W0 + c0) * DIM,
                          ap=[[DIM, vc], [1, DIM]])
            eng.dma_start(dst[r * 14:r * 14 + vc, :], src)


def _scatter_chunk(nc, dram, src, w, c, eng=None):
    """DMA the valid tokens of half-chunk (w, c) from src [98,768] SBUF back
    to token-major [4096,768] DRAM."""
    eng = eng or nc.sync
    r0, c0, vr, vc = _chunk_geom(w, c)
    if vc == 14:
        dst = bass.AP(tensor=dram, offset=(r0 * W0 + c0) * DIM,
                      ap=[[W0 * DIM, vr], [DIM, 14], [1, DIM]])
        eng.dma_start(dst, src[0:vr * 14, :])
    else:
        for r in range(vr):
            dst = bass.AP(tensor=dram, offset=((r0 + r) * W0 + c0) * DIM,
                          ap=[[DIM, vc], [1, DIM]])
            eng.dma_start(dst, src[r * 14:r * 14 + vc, :])


def build_program():
    nc = bacc.Bacc(None, target_bir_lowering=False, debug=False)

    x_d = nc.dram_tensor("x", [NTOK, DIM], BF16, kind="ExternalInput")
    qkvw_d = nc.dram_tensor("qkvw", [DIM, 3 * DIM], F8, kind="ExternalInput")
    qkvb_d = nc.dram_tensor("qkvb", [2 * DIM], F32, kind="ExternalInput")
    projw_d = nc.dram_tensor("projw", [DIM, DIM], F8, kind="ExternalInput")
    projb_d = nc.dram_tensor("projb", [DIM], F32R, kind="ExternalInput")
    bt_d = nc.dram_tensor("bt", [HC, HEADS, 2, WW], BF16, kind="ExternalInput")
    sel_d = nc.dram_tensor("sel", [HEADS, 6 * 128], BF16, kind="ExternalInput")
    e16_d = nc.dram_tensor("e16", [128, 2 * HEADS * 128], F8, kind="ExternalInput")
    fc1w_d = nc.dram_tensor("fc1w", [DIM, MLP_H], F8, kind="ExternalInput")
    fc1b_d = nc.dram_tensor("fc1b", [MLP_H], F32, kind="ExternalInput")
    fc2w_d = nc.dram_tensor("fc2w", [MLP_H, DIM], F8, kind="ExternalInput")
    fc2b_d = nc.dram_tensor("fc2b", [DIM], F32R, kind="ExternalInput")

    out_d = nc.dram_tensor("out", [NTOK, DIM], F32, kind="ExternalOutput")
    x2_d = nc.dram_tensor("x2", [NTOK, DIM], BF16)  # internal scratch

    with tile.TileContext(nc) as tc:
        with ExitStack() as g:
            # ---------------- global constants / weights ----------------
            consts = g.enter_context(tc.tile_pool(name="consts", bufs=1))
            ident32 = consts.tile([128, 128], F32)
            make_identity(nc, ident32)
            identb = consts.tile([128, 128], BF16)
            nc.vector.tensor_copy(identb[:, :], ident32[:, :])
            ones32 = consts.tile([33, 128], F32)
            nc.vector.memset(ones32[:, :], 1.0)
            ones_r = consts.tile([33, 128], F32R)
            nc.vector.tensor_copy(ones_r[:, :], ones32[:, :])
            e16 = consts.tile([128, 2, HEADS, 128], F8)
            nc.gpsimd.dma_start(e16[:, :, :, :], e16_d[:, :]
                                .rearrange("p (a h m) -> p a h m", a=2, h=HEADS))
            eps_t = consts.tile([128, 1], F32)
            nc.vector.memset(eps_t[:, :], EPS)
            qkvb_sb = consts.tile([128, 12], F32)
            nc.sync.dma_start(
                qkvb_sb[:, :],
                bass.AP(tensor=qkvb_d, offset=0, ap=[[1, 128], [128, 12]]))
            bias2 = consts.tile([33, DIM], F32R)
            nc.sync.dma_start(bias2[0:1, :],
                              bass.AP(tensor=projb_d, offset=0, ap=[[1, DIM]]))
            nc.sync.dma_start(bias2[32:33, :],
                              bass.AP(tensor=fc2b_d, offset=0, ap=[[1, DIM]]))
            fc1b_sb = consts.tile([128, 24], F32)
            nc.sync.dma_start(
                fc1b_sb[:, :],
                bass.AP(tensor=fc1b_d, offset=0, ap=[[1, 128], [128, 24]]))
            sel_sb = consts.tile([HEADS, 6, 128], BF16)
            nc.scalar.dma_start(sel_sb[:, :, :], sel_d[:, :]
                                .rearrange("h (g p) -> h g p", p=128))
            bt_sb = consts.tile([HC, HEADS, 2, WW], BF16)
            for hh in range(3):
                eng = (nc.sync, nc.scalar, nc.gpsimd)[hh]
                eng.dma_start(bt_sb[:, 4 * hh:4 * (hh + 1), :, :],
                              bt_d[:, 4 * hh:4 * (hh + 1), :, :])
            stats2M = consts.tile([128, 32], F32)
            stats2R = consts.tile([128, 32], F32)

            # attention weights (fp8)
            wA = g.enter_context(tc.tile_pool(name="wA", bufs=1))
            qkvw_sb = wA.tile([128, 6, 3 * DIM], F8)
            for kk in range(6):
                for hh in range(2):
                    eng = (nc.sync, nc.scalar, nc.gpsimd)[(2 * kk + hh) % 3]
                    eng.dma_start(
                        qkvw_sb[:, kk, hh * 1152:(hh + 1) * 1152],
                        qkvw_d[kk * 128:(kk + 1) * 128,
                               hh * 1152:(hh + 1) * 1152])
            projw_sb = wA.tile([128, 6, DIM], F8)
            for kk in range(3):
                eng = (nc.sync, nc.scalar, nc.gpsimd)[kk]
                eng.dma_start(
                    projw_sb[:, 2 * kk:2 * kk + 2, :],
                    projw_d[kk * 256:(kk + 1) * 256, :]
                    .rearrange("(a p) n -> p a n", p=128))

            # MLP weights (fp8): tiles allocated now, DMAs emitted
            # after pair 0 so they don't delay the first gathers
            wB = g.enter_context(tc.tile_pool(name="wB", bufs=1))
            fc1w_sb = wB.tile([128, 6, MLP_H], F8)
            fc2w_sb = wB.tile([128, 24, DIM], F8)

            def load_fc_weights():
                for kk in range(6):
                    eng = (nc.sync, nc.scalar, nc.gpsimd)[kk % 3]
                    eng.dma_start(fc1w_sb[:, kk, :],
                                  fc1w_d[kk * 128:(kk + 1) * 128, :])
                for kk in range(8):
                    eng = (nc.sync, nc.scalar, nc.gpsimd)[kk % 3]
                    eng.dma_start(
                        fc2w_sb[:, 3 * kk:3 * (kk + 1), :],
                        fc2w_d[kk * 384:(kk + 1) * 384, :]
                        .rearrange("(a p) n -> p a n", p=128))

            # natural_log_exp_and_others: exp (softmax) + ln/exp (rstd) + gelu
            nc.scalar.add_instruction(mybir.InstLoadActFuncSet(
                name=nc.get_next_instruction_name(), ins=[], outs=[],
                act_func_set_id=6))

            # ---------------- attention over window pairs ----------------
            with ExitStack() as s2:
                pxp = s2.enter_context(tc.tile_pool(name="pxp", bufs=2))
                pln = s2.enter_context(tc.tile_pool(name="pln", bufs=2))
                phT = s2.enter_context(tc.tile_pool(name="phT", bufs=2))
                pqk = s2.enter_context(tc.tile_pool(name="pqk", bufs=2))
                pvT = s2.enter_context(tc.tile_pool(name="pvT", bufs=2))
                vt_alloc = [0]
                pUT = s2.enter_context(tc.tile_pool(name="pUT", bufs=3))
                ut_alloc = [0]
                prd = s2.enter_context(tc.tile_pool(name="prd", bufs=2))
                prr = s2.enter_context(tc.tile_pool(name="prr", bufs=3))
                pOT = s2.enter_context(tc.tile_pool(name="pOT", bufs=2))
                px2 = s2.enter_context(tc.tile_pool(name="px2", bufs=3))
                pstat = s2.enter_context(tc.tile_pool(name="pstat", bufs=3))
                # MLP pools (super-tiles are pumped into the pair loop)
                pxt = s2.enter_context(tc.tile_pool(name="pxt", bufs=7))
                pxn = s2.enter_context(tc.tile_pool(name="pxn", bufs=3))
                pxnx = s2.enter_context(tc.tile_pool(name="pxnx", bufs=5))
                pxnT = s2.enter_context(tc.tile_pool(name="pxnT", bufs=2))
                pgT = s2.enter_context(tc.tile_pool(name="pgT", bufs=2))
                pout = s2.enter_context(tc.tile_pool(name="pout", bufs=2))
                pst = s2.enter_context(tc.tile_pool(name="pst", bufs=2,
                                                   space="PSUM"))
                pps = s2.enter_context(tc.tile_pool(name="pps", bufs=5,
                                                   space="PSUM"))
                psd = s2.enter_context(tc.tile_pool(name="psd", bufs=1,
                                                   space="PSUM"))

                def prep_pair_units(pair, out_holder):
                    """gather + LN1 for a pair as a 4-unit generator, pumped
                    from inside the PREVIOUS pair's stage 1 so the DVE stats
                    work overlaps the ACT exp bursts instead of forming an
                    ACT/DVE convoy. rstd comes from a 4-chunk-batched DVE
                    Newton rsqrt: no ACT table funcs here, so pumped gelus
                    don't thrash the activation table."""
                    x_pair = pxp.tile([HC, 4, DIM], BF16, name="x_pair")
                    ln_pair = pln.tile([HC, 4, DIM], BF16, name="ln_pair")
                    out_holder.append((x_pair, ln_pair))
                    nch_p = 2 * len(pair)
                    mvp = pstat.tile([HC, 4, 2], F32, tag="mv", bufs=2,
                                     name="mvp")
                    for wl, w in enumerate(pair):
                        for c in range(2):
                            i = 2 * wl + c
                            _gather_chunk(nc, x_pair[:, i, :], x_d, w, c,
                                          eng=nc.gpsimd)
                            st = pstat.tile([HC, 3, 6], F32, name="bst")
                            for gg in range(3):
                                nc.vector.bn_stats(
                                    st[:, gg, :],
                                    x_pair[:, i, gg * 256:(gg + 1) * 256])
                            nc.vector.bn_aggr(mvp[:, i, :], st[:, :, :])
                            if i < nch_p - 1:
                                yield
                    # batched rsqrt(var+eps) for all chunks (quake + Newton)
                    nw8 = pstat.tile([HC, 6, 4], F32, tag="nw", bufs=2,
                                     name="nw8")
                    ve = nw8[:, 0, :nch_p]
                    nc.vector.tensor_scalar(
                        out=ve, in0=mvp[:, :nch_p, 1], scalar1=EPS,
                        scalar2=None, op0=OP.add)
                    yi = nw8[:, 1, :nch_p].bitcast(mybir.dt.int32)
                    nc.vector.tensor_scalar(
                        out=yi, in0=ve.bitcast(mybir.dt.int32),
                        scalar1=1, scalar2=None, op0=OP.arith_shift_right)
                    y0 = nw8[:, 2, :nch_p].bitcast(mybir.dt.int32)
                    nc.vector.tensor_scalar(
                        out=y0, in0=yi, scalar1=-1,
                        scalar2=None, op0=OP.bitwise_xor)
                    nc.vector.tensor_scalar(
                        out=y0, in0=y0, scalar1=0x5f3759e0,
                        scalar2=None, op0=OP.add)
                    ya, yb = nw8[:, 2, :nch_p], nw8[:, 3, :nch_p]
                    t2, w5 = nw8[:, 4, :nch_p], nw8[:, 5, :nch_p]
                    for _ in range(3):
                        nc.vector.tensor_tensor(
                            out=t2, in0=ya, in1=ya, op=OP.mult)
                        nc.vector.scalar_tensor_tensor(
                            out=w5, in0=ve, scalar=-0.5, in1=t2,
                            op0=OP.mult, op1=OP.mult)
                        nc.vector.tensor_scalar(
                            out=w5, in0=w5, scalar1=1.5,
                            scalar2=None, op0=OP.add)
                        nc.vector.tensor_tensor(
                            out=yb, in0=ya, in1=w5, op=OP.mult)
                        ya, yb = yb, ya
                    for i in range(nch_p):
                        nc.vector.tensor_scalar(
                            out=ln_pair[:, i, :], in0=x_pair[:, i, :],
                            scalar1=mvp[:, i, 0:1],
                            scalar2=ya[:, i:i + 1],
                            op0=OP.subtract, op1=OP.mult)

                def mlp_st_units(st):
                    """One 512-token MLP super-tile as a lazy unit stream
                    (16 yields): 4x chunk prep (DMA + LN2 stats, no PE),
                    4x transpose, 4x fc1 (6 m-blocks each), 4x fc2 chunk.
                    Units are pumped into the attention pair loop so the
                    in-order engine queues fill attention's bubbles; x2 rows
                    [512*st, 512*st+512) must already be scattered."""
                    x2nT = pxnT.tile([128, 6, 512], F8, name="x2nT")
                    x2t, xns = [], []
                    for c in range(4):
                        t = st * 4 + c
                        xt = pxt.tile([128, DIM], BF16, name="x2t")
                        # never on ACT/DVE: this load WAITS on the x2
                        # scatters, and a DMA's waits hold the issuing
                        # engine's queue
                        eng = nc.gpsimd if t % 2 == 0 else nc.sync
                        eng.dma_start(xt[:, :], x2_d[t * 128:(t + 1) * 128, :])
                        x2t.append(xt)
                        # LN2 stats inline; rsqrt on DVE only
                        # (quake seed + 3 Newton steps)
                        st5 = pxn.tile([128, 3, 6], F32, name="st5")
                        for gg in range(3):
                            nc.vector.bn_stats(
                                st5[:, gg, :],
                                xt[:, gg * 256:(gg + 1) * 256])
                        mv5 = pxn.tile([128, 8], F32, name="mv5")
                        nc.vector.bn_aggr(mv5[:, 0:2], st5[:, :, :])
                        nc.gpsimd.tensor_copy(stats2M[:, t % 32:t % 32 + 1],
                                              mv5[:, 0:1])
                        # LN2 stats inline; rsqrt on DVE only
                        # (quake seed + 3 Newton steps)
                        ve = mv5[:, 2:3]
                        nc.vector.tensor_scalar(
                            out=ve, in0=mv5[:, 1:2], scalar1=EPS,
                            scalar2=None, op0=OP.add)
                        yi = mv5[:, 3:4].bitcast(mybir.dt.int32)
                        nc.vector.tensor_scalar(
                            out=yi, in0=ve.bitcast(mybir.dt.int32),
                            scalar1=1, scalar2=None,
                            op0=OP.arith_shift_right)
                        y0 = mv5[:, 4:5].bitcast(mybir.dt.int32)
                        nc.vector.tensor_scalar(
                            out=y0, in0=yi, scalar1=-1,
                            scalar2=None, op0=OP.bitwise_xor)
                        nc.vector.tensor_scalar(
                            out=y0, in0=y0, scalar1=0x5f3759e0,
                            scalar2=None, op0=OP.add)
                        ya, yb = mv5[:, 4:5], mv5[:, 5:6]
                        t2, w5 = mv5[:, 6:7], mv5[:, 7:8]
                        for _ in range(3):
                            nc.vector.tensor_tensor(
                                out=t2, in0=ya, in1=ya, op=OP.mult)
                            nc.vector.scalar_tensor_tensor(
                                out=w5, in0=ve, scalar=-0.5, in1=t2,
                                op0=OP.mult, op1=OP.mult)
                            nc.vector.tensor_scalar(
                                out=w5, in0=w5, scalar1=1.5,
                                scalar2=None, op0=OP.add)
                            nc.vector.tensor_tensor(
                                out=yb, in0=ya, in1=w5, op=OP.mult)
                            ya, yb = yb, ya
                        nc.gpsimd.tensor_copy(stats2R[:, t % 32:t % 32 + 1], ya)
                        xn = pxnx.tile([128, DIM], BF16, name="x2n")
                        nc.vector.tensor_scalar(
                            out=xn[:, :], in0=xt[:, :],
                            scalar1=stats2M[:, t % 32:t % 32 + 1],
                            scalar2=stats2R[:, t % 32:t % 32 + 1],
                            op0=OP.subtract, op1=OP.mult)
                        xns.append(xn)
                        yield 'f'
                    for c in range(4):
                        for k2 in range(3):
                            ps_t2 = pps.tile([128, 2, 128], BF16, tag="ps",
                                             name="ps_t2")
                            for kk2 in range(2):
                                k = 2 * k2 + kk2
                                nc.tensor.transpose(
                                    ps_t2[:, kk2, :],
                                    xns[c][:, k * 128:(k + 1) * 128],
                                    identb[:, :])
                            # x4: fp8 range scaling for fc1 (w also x4)
                            nc.vector.tensor_scalar(
                                out=x2nT[:, 2 * k2:2 * k2 + 2,
                                         c * 128:(c + 1) * 128],
                                in0=ps_t2[:, :, :],
                                scalar1=4.0, scalar2=None, op0=OP.mult)
                        yield ('g' if c == 3 else 'f')
                    gT = pgT.tile([128, 24, 512], F8, name="gT")
                    for mg in range(4):
                        for m in range(6 * mg, 6 * mg + 6):
                            ps_f1 = pst.tile([128, 512], F32, tag="st",
                                             name="ps_f1")
                            for kk in range(3):
                                nc.tensor.matmul(
                                    ps_f1[:, :],
                                    fc1w_sb[:, 2 * kk:2 * kk + 2,
                                            m * 128:(m + 1) * 128],
                                    x2nT[:, 2 * kk:2 * kk + 2, :],
                                    start=(kk == 0), stop=(kk == 2),
                                    perf_mode=DR)
                            # psum = 16 * preact; gelu(ps/16 + b)
                            nc.scalar.activation(
                                gT[:, m, :], ps_f1[:, :], AF.Gelu,
                                bias=fc1b_sb[:, m:m + 1], scale=1.0 / 16.0)
                        yield ('g' if mg < 3 else 'f')
                    for c in range(4):
                        outt = pout.tile([128, DIM], F32, name="outt")
                        for n in range(2):
                            ps_f2 = pps.tile([128, 384], F32, tag="ps",
                                             name="ps_f2")
                            for m in range(12):
                                nc.tensor.matmul(
                                    ps_f2[:, :384],
                                    gT[:, 2 * m:2 * m + 2,
                                       c * 128:(c + 1) * 128],
                                    fc2w_sb[:, 2 * m:2 * m + 2,
                                            n * 384:(n + 1) * 384],
                                    start=(m == 0), stop=False,
                                    perf_mode=DR)
                            nc.tensor.matmul(
                                ps_f2[:, :384],
                                ones_r[32:33, :128],
                                bias2[32:33, n * 384:(n + 1) * 384],
                                start=False, stop=True)
                            nc.vector.scalar_tensor_tensor(
                                out=outt[:, n * 384:(n + 1) * 384],
                                in0=ps_f2[:, :384],
                                scalar=1.0 / 16.0,
                                in1=x2t[c][:, n * 384:(n + 1) * 384],
                                op0=OP.mult, op1=OP.add)
                        t = st * 4 + c
                        nc.gpsimd.dma_start(
                            out_d[t * 128:(t + 1) * 128, :], outt[:, :])
                        yield 'f'

                # super-tile st readable once the pair covering its last
                # image row has scattered: rows 8st..8st+7 -> window row
                # (8st+7)//14 -> last window 5*wrow+4 -> pair (5*wrow+4)//2
                UNLOCK = {2: [0], 4: [1, 2], 7: [3, 4], 9: [5, 6], 12: [7]}
                mlp_q = []   # entries [generator, kind-of-next-unit]

                def pump(n, want=None):
                    while mlp_q and n > 0:
                        g, kind = mlp_q[0]
                        if want is not None and kind != want:
                            return
                        try:
                            mlp_q[0][1] = next(g)
                            n -= 1
                        except StopIteration:
                            mlp_q.pop(0)

                prepq = []
                prep_state = [None]

                def pump_prep(n=1):
                    while prep_state[0] is not None and n > 0:
                        try:
                            next(prep_state[0])
                        except StopIteration:
                            prep_state[0] = None
                        n -= 1

                def start_prep(pair):
                    prep_state[0] = prep_pair_units(pair, prepq)
                    pump_prep(1)

                start_prep(PAIRS[0])
                pump_prep(8)
                for pi, pair in enumerate(PAIRS):
                    nw = len(pair)
                    PW = nw * WW          # 392 or 196
                    nch = 2 * nw          # chunks in pair
                    x_pair, ln_pair = prepq.pop(0)

                    # transpose ln (bf16) -> evac casts to fp8 hT [128, 6, PW]
                    # (padded to 512 cols: vT stationary slabs read 128-wide
                    # at 98-strides; tail is zeroed, extra rows discarded)
                    hT = phT.tile([128, 6, 512], F8, name="hT")
                    nc.gpsimd.memset(hT[:, :, PW:], 0.0)
                    for k in range(6):
                        ps_t = pps.tile([128, 392], BF16, tag="ps", name="ps_t")
                        for i in range(nch):
                            nc.tensor.transpose(
                                ps_t[:, i * HC:(i + 1) * HC],
                                ln_pair[:HC, i, k * 128:(k + 1) * 128],
                                identb[:HC, :HC])
                        nc.vector.tensor_copy(hT[:, k, :PW], ps_t[:, :PW])
                    pump(1)

                    # fused q,k features [1536, PW] via DoubleRow fp8.
                    # Emit interleaved (0,6,1,7,..) so head h's operands
                    # finish earliest.
                    qkT = pqk.tile([128, 12, 2 * WW], F8, name="qkT")
                    for m in [0, 6, 1, 7, 2, 8, 3, 9, 4, 10, 5, 11]:
                        ps_qk = pps.tile([128, 392], F32, tag="ps", name="ps_qk")
                        for kk in range(3):
                            nc.tensor.matmul(
                                ps_qk[:, :PW],
                                qkvw_sb[:, 2 * kk:2 * kk + 2,
                                        m * 128:(m + 1) * 128],
                                hT[:, 2 * kk:2 * kk + 2, :PW],
                                start=(kk == 0), stop=(kk == 2),
                                perf_mode=DR)
                        if m % 3 == 2:
                            nc.vector.tensor_scalar(
                                out=qkT[:, m, :PW], in0=ps_qk[:, :PW],
                                scalar1=qkvb_sb[:, m:m + 1], scalar2=None,
                                op0=OP.add)
                        else:
                            nc.scalar.activation(
                                qkT[:, m, :PW], ps_qk[:, :PW],
                                AF.Identity, bias=qkvb_sb[:, m:m + 1])
                    pump(2)

                    # v token-major fp8, checkerboard layout [128, chunk, 2
                    # K-slabs, 768]: even heads' 64-feature blocks in slab 0,
                    # odd heads' in slab 1; complementary halves and rows
                    # 98:128 stay zero (memset once per ring buffer). AV then
                    # contracts a head-pair in DoubleRow matmuls whose
                    # 128-row outputs start at partition 0 (ISA quadrant rule
                    # forbids DR dst at partition 64).
                    vT = pvT.tile([128, 4, 2, DIM], F8, name="vT")
                    if vt_alloc[0] < 2:
                        nc.gpsimd.memset(vT[:, :, :, :], 0.0)
                        vt_alloc[0] += 1
                    for i in range(nch):
                        for n in range(2):
                            ps_v = pps.tile([128, 384], F32, tag="ps", name="ps_v")
                            for kk in range(3):
                                nc.tensor.matmul(
                                    ps_v[:, :384],
                                    hT[:, 2 * kk:2 * kk + 2, i * HC:i * HC + 128],
                                    qkvw_sb[:, 2 * kk:2 * kk + 2,
                                            2 * DIM + n * 384:2 * DIM + (n + 1) * 384],
                                    start=(kk == 0), stop=(kk == 2),
                                    perf_mode=DR)
                            # col c -> slab (c//64)%2, feature col c
                            dst = bass.AP(
                                tensor=vT.tensor,
                                offset=i * 2 * DIM + n * 384,
                                ap=[[vT.ap[0][0], HC],
                                    [128, 3], [DIM + 64, 2], [1, 64]])
                            nc.vector.tensor_copy(dst, ps_v[:HC, :384])

                    if pi + 1 < len(PAIRS):
                        start_prep(PAIRS[pi + 1])
                    if pi == 0:
                        load_fc_weights()
                    pump(2)

                    OT = pOT.tile([128, 6, 512], F8, name="OT")
                    nc.gpsimd.memset(OT[:, :, PW:], 0.0)

                    # stage 1 (window-outer so only one den group is pending
                    # per bank): S^T + 128*B -> exp -> U^T fp8 -> den.
                    # U^T for all heads of a window lives in one tile
                    # [128, 12, 2, WW] so AV can slice two adjacent heads as
                    # DoubleRow K-slabs.
                    rden = prd.tile([HEADS, 2 * WW], BF16, name="rden")
                    UTw = [None] * nw
                    for wl in range(nw):
                        ps_den = psd.tile([128, 512], F32, tag="psd",
                                          name="ps_den")
                        UT = pUT.tile([128, HEADS, 2, WW], F8, name="UT")
                        UTw[wl] = UT
                        if ut_alloc[0] < 3:
                            # whole tile zeroed once per ring buffer: rows
                            # 98:128 must stay zero for the K=128 DoubleRow
                            # den/AV, and pad-q columns of edge windows are
                            # never written by exp (skipped), so they must
                            # start finite
                            nc.gpsimd.memset(UT[:, :, :, :], 0.0)
                            ut_alloc[0] += 1

                        def emit_den(h):
                            # den (x16): K=128/M=128 DoubleRow over both kc
                            # chunks at once (rows >= 98 of e16/UT are zero).
                            # skip_group_check: the group legitimately stays
                            # open across all heads; the interp's conservative
                            # region marking false-positives on other banks.
                            nc.tensor.matmul(
                                ps_den[:, :WW],
                                e16[:, :, h, :],
                                UT[:, h, :, :],
                                start=(h == 0), stop=(h == HEADS - 1),
                                perf_mode=DR,
                                skip_group_check=True)

                        # pad-query skip for edge windows: scores/bias only
                        # stream the valid q columns (packed in ps_S), exp
                        # scatters them back to window-coord UT positions.
                        # Pad-q lanes of UT/den/OT carry garbage that only
                        # ever reaches unscattered pad-token rows.
                        w_abs = pair[wl]
                        wwi, wwj = divmod(w_abs, NW)

                        def q_in(ap196):
                            """valid-q view of an AP whose last dim is WW"""
                            if wwj == 4:
                                v = ap196.rearrange("p (r c) -> p r c", c=WIN)
                                return (v[:, 0:8, 0:8] if wwi == 4
                                        else v[:, :, 0:8])
                            if wwi == 4:
                                return ap196[:, 0:112]
                            return ap196

                        NQ = (64 if (wwi == 4 and wwj == 4) else
                              112 if (wwi == 4 or wwj == 4) else WW)
                        packed = wwj == 4   # valid q non-contiguous: pack

                        prev = None
                        for h in range(HEADS):
                            qrow = (h % 2) * 64
                            qm = h // 2
                            km = 6 + h // 2
                            ps_S = pst.tile([128, 512], F32, tag="st",
                                            name="ps_S")
                            for kc in range(2):
                                if packed:
                                    s_out = ps_S[:HC, kc * NQ:(kc + 1) * NQ]
                                else:
                                    s_out = ps_S[:HC, kc * WW:kc * WW + NQ]
                                nc.tensor.matmul(
                                    s_out,
                                    qkT[qrow:qrow + 64, km,
                                        (2 * wl + kc) * HC:(2 * wl + kc + 1) * HC],
                                    q_in(qkT[qrow:qrow + 64, qm,
                                             wl * WW:(wl + 1) * WW]),
                                    start=True, stop=False)
                                nc.tensor.matmul(
                                    s_out,
                                    identb[:HC, :HC],
                                    q_in(bt_sb[:, h, kc, :]),
                                    start=False, stop=True)
                            # den of head h-1 here: keeps the PE stream from
                            # stalling on exp(h) before scores(h+1)
                            if prev is not None:
                                emit_den(prev)
                            if packed:
                                e_in = (ps_S[:HC, 0:2 * NQ]
                                        .rearrange("p (k r c) -> p k r c",
                                                   k=2, c=8))
                                ut_slice = UT[:HC, h, :, :].rearrange(
                                    "p k (r c) -> p k r c", c=WIN)
                                e_out = (ut_slice[:, :, 0:8, 0:8]
                                         if wwi == 4
                                         else ut_slice[:, :, :, 0:8])
                            elif NQ < WW:
                                e_in = (ps_S[:HC, 0:2 * WW]
                                        .rearrange("p (k q) -> p k q", k=2)
                                        [:, :, 0:NQ])
                                e_out = UT[:HC, h, :, 0:NQ]
                            else:
                                e_in = ps_S[:HC, 0:2 * WW]
                                e_out = UT[:HC, h, :, :]
                            nc.scalar.activation(
                                e_out, e_in,
                                AF.Exp, scale=1.0 / 128.0)
                            prev = h
                            if h in (2, 5, 8):
                                pump(2, want='f')
                            elif h == 9:
                                pump_prep(1)
                        emit_den(prev)
                        with nc.allow_low_precision("1/den in bf16: 0.4% on "
                                                    "a normalizer, under fp8 "
                                                    "operand noise"):
                            nc.vector.reciprocal(
                                rden[:, wl * WW:(wl + 1) * WW],
                                ps_den[:HEADS, :WW])
                        pump(1, want='f')
                        pump_prep(1)

                    # stage 2, fused per head-pair j = (2j, 2j+1): both
                    # heads land in one [128, PW] PSUM tile (rows 0-63 /
                    # 64-127) via checkerboard-vT DoubleRow matmuls,
                    # normalized by one combined rdrep in one DVE op.
                    def pair_av(j, rdrep):
                        ps_O = pps.tile([128, 392], F32, tag="ps", name="ps_O")
                        for wl in range(nw):
                            for kc in range(2):
                                nc.tensor.matmul(
                                    ps_O[:, wl * WW:(wl + 1) * WW],
                                    vT[:, 2 * wl + kc, :,
                                       j * 128:(j + 1) * 128],
                                    UTw[wl][:, 2 * j:2 * j + 2, kc, :],
                                    start=(kc == 0), stop=(kc == 1),
                                    perf_mode=DR)
                        nc.vector.tensor_tensor(
                            out=OT[:, j, :PW],
                            in0=ps_O[:, :PW],
                            in1=rdrep[:, :PW],
                            op=OP.mult)

                    # rep(j+1) emitted before AV(j): PE never stalls on the
                    # ACT rdrep copy
                    def emit_rep(j):
                        ps_rep = pps.tile([128, 392], F32, tag="ps",
                                          name="ps_rep")
                        nc.tensor.matmul(
                            ps_rep[:, :PW],
                            sel_sb[:, j, :],
                            rden[:, :PW],
                            start=True, stop=True)
                        rdrep = prr.tile([128, 2 * WW], BF16, name="rdrep")
                        nc.scalar.copy(rdrep[:, :PW], ps_rep[:, :PW])
                        return rdrep

                    rdrep_cur = emit_rep(0)
                    for j in range(6):
                        rdrep_nxt = emit_rep(j + 1) if j + 1 < 6 else None
                        pair_av(j, rdrep_cur)
                        rdrep_cur = rdrep_nxt
                    pump(2)

                    # proj (DR fp8) + 1/16 + residual -> x2 bf16, scatter
                    for i in range(nch):
                        x2c = px2.tile([HC, DIM], BF16, name="x2c")
                        for n in range(2):
                            ps_pr = pps.tile([128, 384], F32, tag="ps", name="ps_pr")
                            for kk in range(3):
                                nc.tensor.matmul(
                                    ps_pr[:, :384],
                                    OT[:, 2 * kk:2 * kk + 2, i * HC:i * HC + 128],
                                    projw_sb[:, 2 * kk:2 * kk + 2,
                                             n * 384:(n + 1) * 384],
                                    start=(kk == 0), stop=False,
                                    perf_mode=DR)
                            nc.tensor.matmul(
                                ps_pr[:, :384],
                                ones_r[0:1, :128],
                                bias2[0:1, n * 384:(n + 1) * 384],
                                start=False, stop=True)
                            nc.vector.scalar_tensor_tensor(
                                out=x2c[:, n * 384:(n + 1) * 384],
                                in0=ps_pr[:HC, :384],
                                scalar=1.0 / 16.0,
                                in1=x_pair[:, i, n * 384:(n + 1) * 384],
                                op0=OP.mult, op1=OP.add)
                        _scatter_chunk(nc, x2_d, x2c[:, :], pair[i // 2], i % 2,
                                       eng=nc.sync)
                    pump_prep(8)
                    pump(2)

                    for st in UNLOCK.get(pi, []):
                        mlp_q.append([mlp_st_units(st), 'f'])

                # drain remaining MLP units
                pump(1 << 30)

    nc.compile()
    return nc


_NC_CACHE = {}


def _get_nc():
    if "nc" not in _NC_CACHE:
        _NC_CACHE["nc"] = build_program()
    return _NC_CACHE["nc"]


def _prep_weights(inputs):
    f = lambda k: np.asarray(inputs[k], np.float32)
    x = f("x")
    ln1_g, ln1_b = f("ln1_g"), f("ln1_b")
    ln2_g, ln2_b = f("ln2_g"), f("ln2_b")
    qkv_w, qkv_b = f("qkv_w"), f("qkv_b")
    proj_w, proj_b = f("proj_w"), f("proj_b")
    fc1_w, fc1_b = f("fc1_w"), f("fc1_b")
    fc2_w, fc2_b = f("fc2_w"), f("fc2_b")
    rel = f("rel_pos_bias")
    SCALE = np.float32((DIM // HEADS) ** -0.5)

    # fold ln1 into qkv
    qkv_w_f = qkv_w * ln1_g[None, :]
    qkv_b_f = qkv_w @ ln1_b + qkv_b
    # fold v bias into proj bias (softmax rows sum to 1)
    projb = proj_b + proj_w @ qkv_b_f[2 * DIM:]
    # fp8 scaling: q rows *(8*SCALE)=1.0, k rows *16, v rows *16
    qkv_w_s = qkv_w_f.copy()
    qkv_w_s[:DIM] *= 8.0 * SCALE
    qkv_w_s[DIM:] *= 16.0
    qkv_b_s = qkv_b_f[:2 * DIM].copy()
    qkv_b_s[:DIM] *= 8.0 * SCALE
    qkv_b_s[DIM:] *= 16.0
    # proj: *16 (OT is true-scale); output scaled back by 1/16 in DVE
    projw_s = proj_w * 16.0
    projb_s = projb * 16.0
    # fold ln2 into fc1
    fc1_w_f = fc1_w * ln2_g[None, :]
    fc1_b_f = fc1_w @ ln2_b + fc1_b

    # 128*B transposed: bt[p, h, kc, q] = 128*rel[h, q, kc*98+p]
    bt = (128.0 * rel).transpose(0, 2, 1)          # [h, k, q]
    bt = bt.reshape(HEADS, 2, HC, WW).transpose(2, 0, 1, 3).copy()

    # sel[h', j, p] = 1 where h' = 2j (p<64) or 2j+1 (p>=64): one
    # replicate-matmul builds both heads' 1/den rows of an OT m-chunk
    sel = np.zeros((HEADS, 6, 128), np.float32)
    for j in range(6):
        sel[2 * j, j, :64] = 1.0
        sel[2 * j + 1, j, 64:] = 1.0
    sel = sel.reshape(HEADS, 6 * 128)

    e16 = np.zeros((128, 2, HEADS, 128), np.float32)
    for h in range(HEADS):
        e16[:HC, :, h, h] = 16.0
    e16 = e16.reshape(128, 2 * HEADS * 128)

    e4 = ml_dtypes.float8_e4m3
    b16 = ml_dtypes.bfloat16
    return {
        "qkvw": np.ascontiguousarray(qkv_w_s.T).astype(e4),   # [768, 2304]
        "qkvb": qkv_b_s,
        "projw": np.ascontiguousarray(projw_s.T).astype(e4),  # [768, 768]
        "projb": projb_s,
        "bt": bt.astype(b16),
        "sel": sel.astype(b16),
        "e16": e16.astype(e4),
        "fc1w": np.ascontiguousarray(fc1_w_f.T * 4.0).astype(e4),  # [768, 3072]
        "fc1b": fc1_b_f,
        "fc2w": np.ascontiguousarray(fc2_w.T * 16.0).astype(e4),  # [3072, 768]
        "fc2b": fc2_b * 16.0,
    }, x


PROFILE = False
LAST_RESULT = None


def prepare_exec(inputs):
    """bench.py hook: (nc, per-core in_maps, n_cores)."""
    weights, x = _prep_weights(inputs)
    nc = _get_nc()
    in_maps = [dict(weights, x=np.ascontiguousarray(x[i]).astype(ml_dtypes.bfloat16))
               for i in range(B)]
    return nc, in_maps, B


def kernel(**inputs):
    global LAST_RESULT
    weights, x = _prep_weights(inputs)
    nc = _get_nc()
    in_maps = [dict(weights, x=np.ascontiguousarray(x[i]).astype(ml_dtypes.bfloat16))
               for i in range(B)]
    res = run_bass_kernel_spmd(nc, in_maps, core_ids=list(range(B)),
                               trace=PROFILE)
    LAST_RESULT = res
    out = np.stack([res.results[i]["out"] for i in range(B)], axis=0)
    return out.astype(np.float32)


if __name__ == "__main__":
    rng = np.random.default_rng(0)
    ins = {
        "x": rng.standard_normal((B, NTOK, DIM), dtype=np.float32),
        "rel_pos_bias": (rng.standard_normal((HEADS, WW, WW)) * 0.1).astype(np.float32),
        "ln1_g": np.ones(DIM, np.float32), "ln1_b": np.zeros(DIM, np.float32),
        "qkv_w": (rng.standard_normal((3 * DIM, DIM)) * 0.02).astype(np.float32),
        "qkv_b": np.zeros(3 * DIM, np.float32),
        "proj_w": (rng.standard_normal((DIM, DIM)) * 0.02).astype(np.float32),
        "proj_b": np.zeros(DIM, np.float32),
        "ln2_g": np.ones(DIM, np.float32), "ln2_b": np.zeros(DIM, np.float32),
        "fc1_w": (rng.standard_normal((MLP_H, DIM)) * 0.02).astype(np.float32),
        "fc1_b": np.zeros(MLP_H, np.float32),
        "fc2_w": (rng.standard_normal((DIM, MLP_H)) * 0.02).astype(np.float32),
        "fc2_b": np.zeros(DIM, np.float32),
        "H": np.int64(64), "W": np.int64(64),
    }
    out = kernel(**ins)
    print("out", out.shape, out.dtype, np.abs(out).max())

